# revision 1
# baseline (speedup 1.0000x reference)
"""Trainium2 Bass kernel for nn_Attention_C (XCA-style channel attention block).

Pipeline per image: 1x1 conv (GEMM) -> depthwise 3x3 conv -> per-head
l2norm + channel-attention (48x48 Gram over 4096 pixels) -> softmax ->
attn @ v -> 1x1 out-projection.

Sharding: data-parallel over batch. 16 images / 8 cores = 2 images per core.
Weights replicated; outputs gathered on host.

Layouts on device (per core):
  - channel-major [chan(partition), pixel(free)] for GEMMs + depthwise conv
  - depthwise conv runs on a w-padded pixel layout (row stride 66, 66x66
    total) so all 9 taps are plain free-dim offset views; work is split
    across TensorE (diagonal-weight matmuls), VectorE and GpSimd
    (scalar_tensor_tensor fused multiply-accumulate chains)
  - q,k are DMA-transposed to pixel-major bf16 for the per-head Gram
    (contraction over pixels on the PE), softmax is batched across heads,
    attn@v uses ctile-partitioned block lhsT matrices assembled from
    PE-transposed attention matrices.
"""

import os
import sys
import types

import numpy as np

_REPO = "/opt/trn_rl_repo"
if _REPO not in sys.path:
    sys.path.insert(0, _REPO)

# ---------------------------------------------------------------------------
# antenv.axon_hooks shim (the image's antenv lacks it; needed for trace=True)
# ---------------------------------------------------------------------------
if "antenv.axon_hooks" not in sys.modules:
    try:
        from trn_agent_boot.trn_boot import _ntff_profile_via_ctypes

        _hook = _ntff_profile_via_ctypes("/opt/axon/libaxon_pjrt.so")
    except Exception:
        _hook = None
    _m = types.ModuleType("antenv.axon_hooks")
    _m.get_axon_ntff_profile_hook = lambda: _hook
    _m.set_axon_ntff_profile_hook = lambda h: None
    sys.modules["antenv.axon_hooks"] = _m

import ml_dtypes  # noqa: E402
import bass_rust  # noqa: E402
import concourse.bass as bass  # noqa: E402
import concourse.mybir as mybir  # noqa: E402
import concourse.tile as tile  # noqa: E402
from concourse.bass_utils import run_bass_kernel_spmd  # noqa: E402
from concourse.masks import make_identity  # noqa: E402

BF16 = mybir.dt.bfloat16
F32 = mybir.dt.float32
AF = mybir.ActivationFunctionType
ALU = mybir.AluOpType
AX = mybir.AxisListType

# ---------------------------------------------------------------------------
# Patch TileContext._drain_and_barrier: this walrus build rejects >1 sync
# waits on a CTRL-class (Drain) instruction; split them into standalone waits.
# ---------------------------------------------------------------------------
_MAX_DRAIN_WAITS = 1


def _split_drain_and_barrier(self, tick_clock, wait_clock):
    from concourse.tile import ScopedClock

    nc = self.nc
    drain_inst = nc.sync.drain()
    wait_clock.add_sem_waits(
        drain_inst.ins, ScopedClock({None: tick_clock.global_clock})
    )
    waits = list(drain_inst.ins.sync_info.on_wait)
    if len(waits) > _MAX_DRAIN_WAITS:
        assert self.sems is not None
        by_num = {h.num: h for h in self.sems.allocated().values()}
        keep, spill = [], []
        for w in waits:
            if w.sync_type == "semaphore" and w.id in by_num:
                spill.append(w)
            else:
                keep.append(w)
        while spill and len(keep) < _MAX_DRAIN_WAITS:
            keep.append(spill.pop())
        drain_inst.ins.sync_info = bass_rust.SyncInfo(on_wait=keep, on_update=[])
        for w in spill:
            nc.sync.wait_ge(by_num[w.id], int(w.wait_value))

    nc.all_engine_barrier()
    assert self.sems is not None
    popped = nc._tile_sem_poison_stack.pop()
    assert popped is self._sem_poison
    nc.clear_and_free_semaphores(list(self.sems.allocated().values()))
    nc.all_engine_barrier()


tile.TileContext._drain_and_barrier = _split_drain_and_barrier


def _split_sync_waits(nc, max_waits=1, max_updates=1):
    """walrus rejects instructions with too many sync wait/update commands;
    spill excess waits onto preceding same-engine NoOps (and excess updates
    onto following ones)."""
    for f in nc.m.functions:
        for bb in f.blocks:
            il = list(bb.instructions)
            out = []
            changed = False
            for inst in il:
                si = inst.sync_info
                if si is None:
                    out.append(inst)
                    continue
                waits = list(si.on_wait)
                ups = list(si.on_update)
                pre, post = [], []
                if len(waits) > max_waits:
                    keep = waits[:max_waits]
                    for i in range(max_waits, len(waits), max_waits):
                        n = mybir.InstNoOp(
                            name=f"I-sw{nc.next_id()}", ins=[], outs=[])
                        n.engine = inst.engine
                        n.sync_info = bass_rust.SyncInfo(
                            on_wait=waits[i : i + max_waits], on_update=[])
                        pre.append(n)
                    changed = True
                else:
                    keep = waits
                if len(ups) > max_updates:
                    kup = ups[:max_updates]
                    for i in range(max_updates, len(ups), max_updates):
                        n = mybir.InstNoOp(
                            name=f"I-su{nc.next_id()}", ins=[], outs=[])
                        n.engine = inst.engine
                        n.sync_info = bass_rust.SyncInfo(
                            on_wait=[], on_update=ups[i : i + max_updates])
                        post.append(n)
                    changed = True
                else:
                    kup = ups
                if pre or post:
                    inst.sync_info = bass_rust.SyncInfo(
                        on_wait=keep, on_update=kup)
                out.extend(pre)
                out.append(inst)
                out.extend(post)
            if changed:
                bb.instructions = out

# ---------------------------------------------------------------------------
# Problem constants (hardcoded; spec: x [16, 384, 64, 64] f32, 8 heads)
# ---------------------------------------------------------------------------
NCORES = 8
BTOT, C, H, W = 16, 384, 64, 64
HEADS = 8
CP = C // HEADS  # 48
C3 = 3 * C  # 1152
NPIX = H * W  # 4096
B = BTOT // NCORES  # images per core

P = 128
RS = W + 2  # padded row stride 66
PADLEN = RS * (H + 2)  # 4356
INT0 = RS + 1  # first real-pixel position 67
INTLEN = RS * H - 2  # 4222 (contiguous span covering all real pixels)
HALF = (INTLEN + 1) // 2  # 2111 BUT we want a row-aligned split:
HALF_A = 32 * RS - 1  # 2111: rows 0..31 (+ trailing pads)
HALF_B = INTLEN - HALF_A  # 2111: rows 32..63
NST = C3 // P  # 9 channel subtiles of qkv
NTN = NPIX // 512  # 8 pixel tiles of 512
KT = NPIX // P  # 32 gram contraction tiles

# conv taps: offset in padded layout, index into W_dw[:, 0, kh, kw]
TAPS = [(RS * (kh - 1) + (kw - 1), kh, kw) for kh in range(3) for kw in range(3)]

# conv work assignment: per (img, st, half) -> engine, tuned for balance
# PE-diag ~7.9us/half, DVE STT ~19.8us/half, GPSIMD STT ~41.8us/half


def _conv_assignment():
    # half A of each subtile on DVE, half B on PE (plus st0 half B on DVE)
    # so both engines stream concurrently within every group; PE stays the
    # busier engine but closer to balance with DVE
    a = {}
    for img in range(B):
        for st in range(NST):
            a[(img, st, 0)] = "dve"
            a[(img, st, 1)] = "dve" if st == 0 else "pe"
    return a


CONV_ASSIGN = _conv_assignment()

# attn@v uses a head-padded channel layout: head h occupies partitions
# 64*(h%2)..+48 of padded ctile h//2 (all boundaries 32-aligned).
NPG = HEADS // 2  # 4 padded ctiles

# head -> list of (partition_lo, partition_hi, subtile) pieces in a [128, 3]
# channel-major layout (for the norm-rearrange DMAs)
def _head_pieces():
    out = {}
    for h in range(HEADS):
        lo, hi = CP * h, CP * h + CP
        pieces = []
        s0, s1 = lo // P, (hi - 1) // P
        for s in range(s0, s1 + 1):
            a = max(lo, P * s) - P * s
            b = min(hi, P * s + P) - P * s
            pieces.append((a, b, s))
        out[h] = pieces
    return out


HEAD_PIECES = _head_pieces()


def _build_nc():
    nc = bass.Bass("TRN2", target_bir_lowering=False, debug=False,
                   num_devices=NCORES)

    # ---- DRAM tensors (host pre-arranged to SBUF-shaped layouts) ----
    x_d = nc.dram_tensor("x", [B, C // P, P, NPIX], BF16, kind="ExternalInput")
    wq_d = nc.dram_tensor("wqT", [P, C // P, C3], BF16, kind="ExternalInput")
    wo_d = nc.dram_tensor("woT", [P, NPG, C], BF16, kind="ExternalInput")
    bq_d = nc.dram_tensor("bq", [P, NST], F32, kind="ExternalInput")
    bdw_d = nc.dram_tensor("bdw", [P, NST], F32, kind="ExternalInput")
    bo_d = nc.dram_tensor("bo", [P, C // P], F32, kind="ExternalInput")
    dww_d = nc.dram_tensor("dww", [P, NST, 9], F32, kind="ExternalInput")
    temp_d = nc.dram_tensor("temp", [CP, HEADS], F32, kind="ExternalInput")
    diag_d = nc.dram_tensor("diag", [NST, 9, P, P], BF16, kind="ExternalInput")
    y_d = nc.dram_tensor("y", [B, C // P, P, NPIX], F32, kind="ExternalOutput")
    n2q_s = nc.dram_tensor("n2q_scratch", [B, P, C // P], F32)
    n2k_s = nc.dram_tensor("n2k_scratch", [B, P, C // P], F32)
    r2_s = nc.dram_tensor("r2_scratch", [B, 1, C], F32)
    v_s = nc.dram_tensor("v_scratch", [B, C, NPIX], BF16)

    with tile.TileContext(nc) as tc:
        with (
            tc.tile_pool(name="consts", bufs=1) as consts,
            tc.tile_pool(name="xt", bufs=3) as xt_pool,
            tc.tile_pool(name="qkvpad", bufs=4) as qkvpad_pool,
            tc.tile_pool(name="convacc", bufs=3) as acc_pool,
            tc.tile_pool(name="convstage", bufs=4) as stage_pool,
            tc.tile_pool(name="qkc", bufs=2) as qkc_pool,
            tc.tile_pool(name="diagw", bufs=2) as diag_pool,
            tc.tile_pool(name="perimg", bufs=1) as perimg,
            tc.tile_pool(name="smalls", bufs=2) as smalls,
            tc.tile_pool(name="attout", bufs=2) as attout_pool,
            tc.tile_pool(name="yt", bufs=2) as yt_pool,
            tc.tile_pool(name="psbig", bufs=6, space="PSUM") as psbig,
            tc.tile_pool(name="psgram", bufs=1, space="PSUM") as psgram,
            tc.tile_pool(name="pstiny", bufs=1, space="PSUM") as pstiny,
        ):
            # ---- load constants ----
            wq = consts.tile([P, C // P, C3], BF16)
            nc.sync.dma_start(out=wq, in_=wq_d[:])
            wo = consts.tile([P, NPG, C], BF16)
            nc.sync.dma_start(out=wo, in_=wo_d[:])
            bq = consts.tile([P, NST], F32)
            nc.sync.dma_start(out=bq, in_=bq_d[:])
            bdw = consts.tile([P, NST], F32)
            nc.sync.dma_start(out=bdw, in_=bdw_d[:])
            bo = consts.tile([P, C // P], F32)
            nc.sync.dma_start(out=bo, in_=bo_d[:])
            dww = consts.tile([P, NST, 9], F32)
            nc.sync.dma_start(out=dww, in_=dww_d[:])
            tempt = consts.tile([CP, HEADS], F32)
            nc.sync.dma_start(out=tempt, in_=temp_d[:])
            ident = consts.tile([P, P], F32)
            make_identity(nc, ident)

            for img in range(B):
                # per-image persistent tiles
                qT = perimg.tile([P, KT, C], BF16, tag="qT")
                kT = perimg.tile([P, KT, C], BF16, tag="kT")
                vpad = perimg.tile([P, NPG, NPIX], BF16, tag="vpad")
                n2q = perimg.tile([P, C // P], F32, tag="n2q")
                n2k = perimg.tile([P, C // P], F32, tag="n2k")

                tdest = {0: qT, 1: kT}

                for grp in range(3):  # 0: q (sts 0-2), 1: k (3-5), 2: v (6-8)
                    sts = [3 * grp + i for i in range(3)]
                    slots = {}
                    for st in sts:
                        slot = qkvpad_pool.tile([P, PADLEN], BF16, tag="qkvpad")
                        slots[st] = slot
                        # zero the pad positions: leading pad row + per-row
                        # 2-elem gaps + final element
                        nc.gpsimd.memset(slot[:, 0:INT0], 0.0)
                        pads = bass.AP(
                            tensor=slot.tensor,
                            offset=slot.offset + (2 * RS - 1),
                            ap=[list(slot.ap[0]), [RS, H - 1], [1, 2]],
                        )
                        nc.gpsimd.memset(pads, 0.0)
                        nc.gpsimd.memset(slot[:, RS * (H + 1) - 1 :], 0.0)

                    # ---- 1x1 qkv GEMM for this group's 3 output ctiles ----
                    for nt in range(NTN):
                        xt = xt_pool.tile([P, C // P, 512], BF16, tag="xt")
                        nc.sync.dma_start(
                            out=xt,
                            in_=x_d[img, :, :, 512 * nt : 512 * nt + 512]
                            .rearrange("k p n -> p k n"),
                        )
                        for st in sts:
                            ps = psbig.tile([P, 512], F32, tag="big")
                            for k in range(C // P):
                                nc.tensor.matmul(
                                    ps,
                                    wq[:, k, P * st : P * st + P],
                                    xt[:, k, :],
                                    start=(k == 0),
                                    stop=(k == C // P - 1),
                                )
                            # strided write into padded layout (8 rows of 64)
                            dest = bass.AP(
                                tensor=slots[st].tensor,
                                offset=slots[st].offset + INT0 + 8 * RS * nt,
                                ap=[list(slots[st].ap[0]), [RS, 8], [1, W]],
                            )
                            nc.scalar.activation(
                                out=dest,
                                in_=ps.rearrange("p (r w) -> p r w", w=W),
                                func=AF.Identity,
                                bias=bq[:, st : st + 1],
                            )

                    # ---- depthwise conv for this group's subtiles ----
                    for st in sts:
                        slot = slots[st]
                        stages = []
                        for hf in range(2):
                            a = INT0 + (0 if hf == 0 else HALF_A)
                            ln = HALF_A if hf == 0 else HALF_B
                            eng = CONV_ASSIGN[(img, st, hf)]
                            stage = stage_pool.tile([P, HALF_A], BF16,
                                                    tag="stage")
                            stages.append((stage, a, ln))
                            if eng == "pe":
                                dm = diag_pool.tile([P, 9, P], BF16, tag="dg")
                                nc.sync.dma_start(
                                    out=dm,
                                    in_=diag_d[st].rearrange("t p q -> p t q"),
                                )
                                off = 0
                                while off < ln:
                                    n = min(512, ln - off)
                                    ps = psbig.tile([P, 512], F32, tag="big")
                                    for t, (toff, kh, kw) in enumerate(TAPS):
                                        nc.tensor.matmul(
                                            ps[:, :n],
                                            dm[:, t, :],
                                            slot[:, a + off + toff :
                                                 a + off + toff + n],
                                            start=(t == 0),
                                            stop=(t == 8),
                                        )
                                    nc.scalar.activation(
                                        out=stage[:, off : off + n],
                                        in_=ps[:, :n],
                                        func=AF.Identity,
                                        bias=bdw[:, st : st + 1],
                                    )
                                    off += n
                            else:
                                e = nc.vector if eng == "dve" else nc.gpsimd
                                acc = acc_pool.tile([P, HALF_A], BF16, tag="acc")
                                for t, (toff, kh, kw) in enumerate(TAPS):
                                    src = slot[:, a + toff : a + toff + ln]
                                    wsc = dww[:, st, t : t + 1]
                                    if t == 0:
                                        e.tensor_scalar(
                                            out=acc[:, :ln], in0=src,
                                            scalar1=wsc,
                                            scalar2=bdw[:, st : st + 1],
                                            op0=ALU.mult, op1=ALU.add,
                                        )
                                    elif t < 8:
                                        e.scalar_tensor_tensor(
                                            out=acc[:, :ln], in0=src,
                                            scalar=wsc, in1=acc[:, :ln],
                                            op0=ALU.mult, op1=ALU.add,
                                        )
                                    else:
                                        e.scalar_tensor_tensor(
                                            out=stage[:, :ln], in0=src,
                                            scalar=wsc, in1=acc[:, :ln],
                                            op0=ALU.mult, op1=ALU.add,
                                        )

                        # ---- compact the padded conv output ----
                        if grp < 2:
                            cdst = qkc_pool.tile([P, NPIX], BF16, tag="qkc")
                        else:
                            cdst = None
                        for hf, (stage, a, ln) in enumerate(stages):
                            # stage holds padded range [a, a+ln); row r of the
                            # half starts at padded 66*(32*hf + r + 1) + 1
                            first_row = 32 * hf
                            row0_off = RS * (first_row + 1) + 1 - a
                            src = bass.AP(
                                tensor=stage.tensor,
                                offset=stage.offset + row0_off,
                                ap=[list(stage.ap[0]), [RS, 32], [1, W]],
                            )
                            if grp < 2:
                                dv = cdst[:, 2048 * hf : 2048 * hf + 2048]
                                nc.gpsimd.dma_start(
                                    out=dv.rearrange("p (r w) -> p r w", w=W),
                                    in_=src,
                                )
                            else:
                                nc.sync.dma_start(
                                    out=v_s[img,
                                            P * (st - 6) : P * (st - 6) + P,
                                            2048 * hf : 2048 * hf + 2048]
                                    .rearrange("p (r w) -> p r w", w=W),
                                    in_=src,
                                )

                        if grp < 2:
                            # pixel-major transpose of this subtile
                            s = st - 3 * grp
                            tq = tdest[grp]
                            nc.sync.dma_start_transpose(
                                tq[:, :, P * s : P * s + P], cdst[:]
                            )
                            # sum of squares per channel
                            n2 = n2q if grp == 0 else n2k
                            nc.scalar.activation(
                                out=cdst[:], in_=cdst[:], func=AF.Square,
                                accum_out=n2[:, s : s + 1],
                            )

                # ---- head-padded v in SBUF ----
                nc.gpsimd.memset(vpad, 0.0)
                for h in range(HEADS):
                    nc.sync.dma_start(
                        out=vpad[64 * (h % 2) : 64 * (h % 2) + CP, h // 2, :],
                        in_=v_s[img, CP * h : CP * h + CP, :],
                    )

                # ---- norms -> head-aligned reciprocal scales ----
                qh2 = smalls.tile([CP, HEADS], F32, tag="qh2")
                r2 = smalls.tile([1, C], F32, tag="r2")
                nc.sync.dma_start(out=n2q_s[img], in_=n2q[:])
                nc.sync.dma_start(out=n2k_s[img], in_=n2k[:])
                for h in range(HEADS):
                    dlo = 0
                    for (a, b, s) in HEAD_PIECES[h]:
                        ln = b - a
                        nc.sync.dma_start(
                            out=qh2[dlo : dlo + ln, h : h + 1],
                            in_=n2q_s[img, a:b, s : s + 1],
                        )
                        nc.sync.dma_start(
                            out=r2[0:1, CP * h + dlo : CP * h + dlo + ln],
                            in_=n2k_s[img, a:b, s : s + 1].rearrange("p o -> o p"),
                        )
                        dlo += ln
                # rq = temp / sqrt(qh2); ck = 1/sqrt(r2)
                rqh = smalls.tile([CP, HEADS], F32, tag="rqh")
                nc.scalar.activation(out=qh2, in_=qh2, func=AF.Sqrt)
                nc.vector.reciprocal(out=qh2, in_=qh2)
                nc.vector.tensor_tensor(out=rqh, in0=qh2, in1=tempt,
                                        op=ALU.mult)
                nc.scalar.activation(out=r2, in_=r2, func=AF.Sqrt)
                nc.vector.reciprocal(out=r2, in_=r2)
                ck = smalls.tile([CP, C], F32, tag="ck")
                nc.sync.dma_start(out=r2_s[img], in_=r2[:])
                nc.sync.dma_start(
                    out=ck,
                    in_=bass.AP(tensor=r2_s, offset=img * C, ap=[[0, CP], [1, C]]),
                )

                # ---- Gram matrices (one head per psum) + row scales ----
                S = smalls.tile([CP, HEADS, CP], F32, tag="S")
                for h in range(HEADS):
                    ps = psgram.tile([CP, CP], F32, tag="gram")
                    for kt in range(KT):
                        nc.tensor.matmul(
                            ps,
                            qT[:, kt, CP * h : CP * h + CP],
                            kT[:, kt, CP * h : CP * h + CP],
                            start=(kt == 0),
                            stop=(kt == KT - 1),
                        )
                    nc.vector.tensor_scalar_mul(
                        S[:, h, :], ps, rqh[:, h : h + 1],
                    )

                # ---- batched softmax over last dim ----
                ckv = ck.rearrange("p (h d) -> p h d", h=HEADS)
                nc.vector.tensor_tensor(out=S, in0=S, in1=ckv, op=ALU.mult)
                mx = smalls.tile([CP, HEADS], F32, tag="mx")
                nc.vector.tensor_reduce(out=mx, in_=S, axis=AX.X, op=ALU.max)
                nc.vector.tensor_tensor(
                    out=S, in0=S, in1=mx[:, :, None].to_broadcast(S.shape),
                    op=ALU.subtract,
                )
                nc.scalar.activation(out=S, in_=S, func=AF.Exp)
                sm = smalls.tile([CP, HEADS], F32, tag="sm")
                nc.vector.tensor_reduce(out=sm, in_=S, axis=AX.X, op=ALU.add)
                nc.vector.reciprocal(out=sm, in_=sm)
                nc.vector.tensor_tensor(
                    out=S, in0=S, in1=sm[:, :, None].to_broadcast(S.shape),
                    op=ALU.mult,
                )

                # ---- transpose attn per head, assemble AV lhsT blocks ----
                lhsav = perimg.tile([P, NPG, P], BF16, tag="lhsav")
                nc.gpsimd.memset(lhsav, 0.0)
                for h in range(HEADS):
                    pst = pstiny.tile([CP, CP], F32, tag="tr")
                    nc.tensor.transpose(pst, S[:, h, :], ident[0:CP, 0:CP])
                    o = 64 * (h % 2)
                    nc.vector.tensor_copy(
                        out=lhsav[o : o + CP, h // 2, o : o + CP], in_=pst,
                    )

                # ---- attn @ v, then out-projection, per pixel tile ----
                for nt in range(NTN):
                    ao = attout_pool.tile([P, NPG, 512], BF16, tag="ao")
                    for g in range(NPG):
                        ps = psbig.tile([P, 512], F32, tag="big")
                        nc.tensor.matmul(
                            ps,
                            lhsav[:, g, :],
                            vpad[:, g, 512 * nt : 512 * nt + 512],
                            start=True,
                            stop=True,
                        )
                        nc.scalar.activation(out=ao[:, g, :], in_=ps,
                                             func=AF.Identity)
                    for mo in range(C // P):
                        ps = psbig.tile([P, 512], F32, tag="big")
                        for k in range(NPG):
                            nc.tensor.matmul(
                                ps,
                                wo[:, k, P * mo : P * mo + P],
                                ao[:, k, :],
                                start=(k == 0),
                                stop=(k == NPG - 1),
                            )
                        yt = yt_pool.tile([P, 512], F32, tag="yt")
                        nc.scalar.activation(
                            out=yt, in_=ps, func=AF.Identity,
                            bias=bo[:, mo : mo + 1],
                        )
                        nc.sync.dma_start(
                            out=y_d[img, mo, :, 512 * nt : 512 * nt + 512],
                            in_=yt,
                        )

    _split_sync_waits(nc)
    return nc


_CACHE = {}


def kernel(x, W_qkv, b_qkv, W_dw, b_dw, W_out, b_out, temperature):
    x = np.asarray(x, np.float32)
    W_qkv = np.asarray(W_qkv, np.float32)
    b_qkv = np.asarray(b_qkv, np.float32)
    W_dw = np.asarray(W_dw, np.float32)
    b_dw = np.asarray(b_dw, np.float32)
    W_out = np.asarray(W_out, np.float32)
    b_out = np.asarray(b_out, np.float32)
    temperature = np.asarray(temperature, np.float32)

    if "nc" not in _CACHE:
        _CACHE["nc"] = _build_nc()
    nc = _CACHE["nc"]

    # ---- host-side prep into SBUF-shaped layouts ----
    wqT = np.ascontiguousarray(
        W_qkv.T.reshape(C // P, P, C3).transpose(1, 0, 2)
    ).astype(ml_dtypes.bfloat16)  # [128, 3, 1152]
    wpad = np.zeros((4 * P, C), np.float32)  # [512, 384] padded in-chans
    for h in range(HEADS):
        wpad[64 * (h % 2) + 128 * (h // 2) : 64 * (h % 2) + 128 * (h // 2) + CP] = \
            W_out.T[CP * h : CP * h + CP]
    woT = np.ascontiguousarray(
        wpad.reshape(4, P, C).transpose(1, 0, 2)
    ).astype(ml_dtypes.bfloat16)  # [128, 4, 384]
    bq = np.ascontiguousarray(b_qkv.reshape(NST, P).T)  # [128, 9]
    bdw = np.ascontiguousarray(b_dw.reshape(NST, P).T)  # [128, 9]
    bo = np.ascontiguousarray(b_out.reshape(C // P, P).T)  # [128, 3]
    taps = W_dw.reshape(C3, 9)  # [1152, 9] in (kh, kw) order
    dww = np.ascontiguousarray(
        taps.reshape(NST, P, 9).transpose(1, 0, 2)
    )  # [128, 9, 9]
    temp = np.broadcast_to(
        temperature.reshape(1, HEADS), (CP, HEADS)
    ).astype(np.float32).copy()  # [48, 8]
    diag = np.zeros((NST, 9, P, P), np.float32)
    ar = np.arange(P)
    for st in range(NST):
        for t in range(9):
            diag[st, t, ar, ar] = taps[P * st : P * st + P, t]
    diag = diag.astype(ml_dtypes.bfloat16)

    xr = x.reshape(BTOT, C // P, P, NPIX).astype(ml_dtypes.bfloat16)  # channel ctile-major per image

    base = {
        "wqT": wqT, "woT": woT, "bq": bq, "bdw": bdw, "bo": bo,
        "dww": dww, "temp": temp, "diag": diag,
    }
    in_maps = []
    for core in range(NCORES):
        m = dict(base)
        m["x"] = np.ascontiguousarray(xr[B * core : B * core + B])
        in_maps.append(m)

    res = run_bass_kernel_spmd(nc, in_maps, list(range(NCORES)),
                               trace=bool(os.environ.get("KERNEL_TRACE")))
    if os.environ.get("KERNEL_TRACE"):
        _CACHE["exec_time_ns"] = res.exec_time_ns

    outs = [res.results[c]["y"].reshape(B, C, H, W) for c in range(NCORES)]
    return np.concatenate(outs, axis=0)

